# revision 1
# baseline (speedup 1.0000x reference)
"""Causal multi-head attention on 8 Trainium2 NeuronCores.

Problem: x[4,2048,1024] @ w_qkv[1024,3072] -> causal MHA (16 heads, hd=64) -> @ w_out.

Sharding: batch (4) x head-group (2 x 8 heads) = 8 cores. Each core:
  phase 1: QKV projection for its batch + its 8 heads.
           Q^T,K^T produced transposed [ch, t] (lhsT = w chunk, rhs = x^T chunk),
           V produced natural [t, ch] (lhsT = x^T chunk, rhs = w_v chunk),
           with a ones column appended per head ([V|1]) for the softmax denominator.
  phase 2: causal attention per head. S^T[k,q] = K_h^T(.T) @ Q_h^T per 128-key block
           (only non-fully-masked blocks computed), additive -1e9 triangular mask on
           diagonal blocks, exp on ACT (softmax max-subtraction skipped: |s/8| <~ 10),
           attn^T[d,q] + denom accumulated in PSUM via lhsT=[V|1], normalized by
           reciprocal + gpsimd partition_broadcast + DVE multiply.
  phase 3: partial output projection out = attn^T.T @ w_out (rows of w_out owned by
           this head group). Host sums the two partials per batch (2-way reduction).

All matmuls run in bf16 (fp32r serialized a ~230ns LDWEIGHTS with every matmul and
kept the PE HAM-throttled at 1.2GHz). Emission is software-pipelined: projection and
output-projection matmul groups are interleaved as fillers inside the attention
chains so the PE stream stays dense (HAM warm) while ACT streams the exps.
Measured ~295us on hardware (max core), rel err 3.5e-3 vs the fp32 reference.
"""
import sys

if "/opt/trn_rl_repo" not in sys.path:
    sys.path.insert(0, "/opt/trn_rl_repo")

import ml_dtypes
import numpy as np

import concourse.tile as tile
from concourse import bacc, mybir
from concourse.bass_utils import run_bass_kernel_spmd

F32 = mybir.dt.float32
F32R = mybir.dt.float32r
BF16 = mybir.dt.bfloat16
EXP = mybir.ActivationFunctionType.Exp

B, T, C, H = 4, 2048, 1024, 16
HD = C // H              # 64
HPC = 8                  # heads per core
CPC = HPC * HD           # 512 channels per core
NCHUNK = C // 128        # 8 contraction chunks of 128
NQ = 4                   # t-quarters (512 each) for phase-1 x streaming
TQ = T // NQ             # 512
NKB = T // 128           # 16 key blocks
NCT = CPC // 128         # 4 c'-tiles per projection (q and k each)

_NC_CACHE = None


def _build_nc():
    """Build the SPMD program (identical on all 8 cores).

    Emission is software-pipelined so the PE instruction stream stays dense
    (HAM stays warm): projection matmul groups for t-quarter tq are interleaved
    between attention chains for q-tile j=tq-1, and output-projection groups
    for quarter tq-1 follow. Within an attention chain, S-block matmuls run two
    kb-pairs ahead of the attention matmuls so exp (ACT) latency is hidden.
    """
    nc = bacc.Bacc()

    wqkv = nc.dram_tensor("wqkv", [NCHUNK, 128, 3 * CPC], BF16, kind="ExternalInput")
    xq = nc.dram_tensor("xq", [NQ, NCHUNK, 128, TQ], BF16, kind="ExternalInput")
    wo = nc.dram_tensor("wo", [NCT, 128, C], BF16, kind="ExternalInput")
    maskneg = nc.dram_tensor("maskneg", [128, 128], BF16, kind="ExternalInput")
    out = nc.dram_tensor("out", [T, C], F32, kind="ExternalOutput")

    with tile.TileContext(nc) as tc, \
         tc.tile_pool(name="pers", bufs=1) as pers, \
         tc.tile_pool(name="xpool", bufs=2) as xpool, \
         tc.tile_pool(name="epool", bufs=8) as epool, \
         tc.tile_pool(name="npool", bufs=4) as npool, \
         tc.tile_pool(name="opool", bufs=4) as opool, \
         tc.tile_pool(name="psum", bufs=1, space="PSUM") as psum:
        # persistent SBUF
        qkt = [pers.tile([128, T], BF16, name=f"qkt{i}") for i in range(2 * NCT)]
        vsb = pers.tile([128, NKB * (CPC + HPC)], BF16, name="vsb")  # 16 x (8 x 65)
        atn = [pers.tile([128, T], BF16, name=f"atn{i}") for i in range(NCT)]
        mask_sb = pers.tile([128, 128], BF16, name="mask_sb")
        nc.sync.dma_start(mask_sb[:], maskneg[:, :])
        # ones columns of [V|1]: memset f32 staging, strided DVE copy (casts to bf16)
        ones_sb = pers.tile([128, NKB * HPC], F32, name="ones_sb")
        nc.vector.memset(ones_sb[:], 1.0)
        nc.vector.tensor_copy(
            vsb.rearrange("p (t h e) -> p (t h) e", h=HPC, e=HD + 1)[:, :, HD:HD + 1],
            ones_sb[:, :, None],
        )
        w_sb = [None] * NCHUNK

        def load_w(c):
            wt = pers.tile([128, 3 * CPC], BF16, name=f"w{c}", uniquify=False)
            nc.sync.dma_start(wt[:], wqkv[c])
            w_sb[c] = wt
        wo_sb = []

        xt_cur = [None] * NCHUNK

        def load_x(tq):
            for c in range(NCHUNK):
                x_t = xpool.tile([128, TQ], BF16, name=f"x{c}", tag=f"x{c}")
                nc.sync.dma_start(x_t[:], xq[tq, c])
                xt_cur[c] = x_t

        def proj_unit(tq, g, xt=None):
            """One projection PSUM group: g in 0..11 (8 QK tiles + 4 V tiles)."""
            if xt is None:
                xt = list(xt_cur)
            if g < 2 * NCT:
                ps = psum.tile([128, TQ], F32, name="psqk", tag="S", bufs=3)
                for c in range(NCHUNK):
                    nc.tensor.matmul(
                        ps[:], w_sb[c][:, 128 * g:128 * (g + 1)], xt[c][:],
                        start=(c == 0), stop=(c == NCHUNK - 1),
                    )
                nc.vector.tensor_copy(qkt[g][:, TQ * tq:TQ * (tq + 1)], ps[:])
            else:
                vt = g - 2 * NCT
                ps = psum.tile([128, CPC], F32, name="psv", tag="S", bufs=3)
                for c in range(NCHUNK):
                    nc.tensor.matmul(
                        ps[:], xt[c][:, 128 * vt:128 * (vt + 1)],
                        w_sb[c][:, 2 * CPC:3 * CPC],
                        start=(c == 0), stop=(c == NCHUNK - 1),
                    )
                ti = tq * (TQ // 128) + vt
                dst = vsb[:, (CPC + HPC) * ti:(CPC + HPC) * (ti + 1)]
                nc.vector.tensor_copy(
                    dst.rearrange("p (h e) -> p h e", e=HD + 1)[:, :, 0:HD],
                    ps.rearrange("p (h e) -> p h e", e=HD),
                )

        def attn_chain(h, j, fillers=None):
            """Causal attention for (head h, 512-query tile j), pipelined:
            S matmuls + exp run LAG pairs ahead of the attention matmuls."""
            ct, r0 = h // 2, HD * (h % 2)
            Qh = qkt[ct][r0:r0 + HD, :]
            Kh = qkt[NCT + ct][r0:r0 + HD, :]
            q0 = 512 * j
            nkb = 4 * j + 4
            npair = (nkb + 1) // 2
            pa = psum.tile([HD + 1, 512], F32, name="pa", tag="A", bufs=2)

            pair_segs = []
            for kbp in range(npair):
                segs, off = [], 0
                for kb in (2 * kbp, 2 * kbp + 1):
                    if kb >= nkb:
                        continue
                    col0 = 0 if kb < 4 * j else 128 * (kb - 4 * j)
                    n = 512 - col0
                    segs.append((kb, col0, n, off))
                    off += n
                pair_segs.append((segs, off))

            ees = [None] * npair

            def emit_s(kbp):
                segs, wsum = pair_segs[kbp]
                ss = psum.tile([128, 1024], F32, name="ss", tag="S", bufs=3)
                ee = epool.tile([128, 1024], BF16, name="ee", tag="E")
                for kb, col0, n, off in segs:
                    nc.tensor.matmul(
                        ss[:, off:off + n],
                        Kh[:, 128 * kb:128 * (kb + 1)],
                        Qh[:, q0 + col0:q0 + 512],
                        start=True, stop=True, skip_group_check=True,
                    )
                nc.scalar.activation(ee[:, 0:wsum], ss[:, 0:wsum], EXP, scale=0.125)
                for kb, col0, n, off in segs:
                    if kb >= 4 * j:  # zero the masked (future) triangle post-exp
                        nc.vector.tensor_mul(
                            ee[:, off:off + 128], ee[:, off:off + 128], mask_sb[:]
                        )
                ees[kbp] = ee

            def emit_a(kbp):
                segs, _ = pair_segs[kbp]
                ee = ees[kbp]
                for kb, col0, n, off in segs:
                    nc.tensor.matmul(
                        pa[:, col0:512],
                        vsb[:, (CPC + HPC) * kb + (HD + 1) * h:
                             (CPC + HPC) * kb + (HD + 1) * (h + 1)],
                        ee[:, off:off + n],
                        start=(kb == 0), stop=(kb == nkb - 1),
                        skip_group_check=True,
                    )

            LAG = 2
            for kbp in range(npair + LAG):
                if kbp < npair:
                    emit_s(kbp)
                if kbp >= LAG:
                    emit_a(kbp - LAG)
                if fillers and kbp % 2 == 1:
                    fillers.pop(0)()
            # normalize by the denominator row
            den = npool.tile([1, 512], F32, name="den", tag="den")
            nc.vector.tensor_copy(den[:], pa[HD:HD + 1, :])
            rec = npool.tile([1, 512], F32, name="rec", tag="rec")
            nc.vector.reciprocal_approx_fast(rec[:], den[:])
            bc = npool.tile([HD, 512], F32, name="bc", tag="bc")
            nc.gpsimd.partition_broadcast(bc[:], rec[:])
            nc.vector.tensor_mul(
                atn[ct][r0:r0 + HD, q0:q0 + 512], pa[0:HD, :], bc[:]
            )

        def outproj_unit(tq, g):
            """One output tile [t128, 512]: g in 0..7 (4 t-tiles x 2 col halves)."""
            tt = tq * 4 + g // 2
            jj = g % 2
            ps = psum.tile([128, 512], F32, name="po", tag="S", bufs=3)
            for cc in range(NCT):
                nc.tensor.matmul(
                    ps[:], atn[cc][:, 128 * tt:128 * (tt + 1)],
                    wo_sb[cc][:, 512 * jj:512 * (jj + 1)],
                    start=(cc == 0), stop=(cc == NCT - 1),
                )
            oc = opool.tile([128, 512], F32, name="oc", tag="oc")
            nc.vector.tensor_copy(oc[:], ps[:])
            nc.sync.dma_start(
                out[128 * tt:128 * (tt + 1), 512 * jj:512 * (jj + 1)], oc[:]
            )

        # ---- pipelined schedule ----
        for c in range(NCHUNK):
            x_t = xpool.tile([128, TQ], BF16, name=f"x{c}", tag=f"x{c}")
            nc.sync.dma_start(x_t[:], xq[0, c])
            xt_cur[c] = x_t
            load_w(c)
        for cc in range(NCT):
            wt = pers.tile([128, C], BF16, name=f"wo{cc}")
            nc.sync.dma_start(wt[:], wo[cc])
            wo_sb.append(wt)
        for g in range(12):
            proj_unit(0, g)
        load_x(1)
        for tq in range(1, NQ + 1):
            j = tq - 1
            fillers = []
            if tq < NQ:
                xts = list(xt_cur)
                for g in range(12):
                    fillers.append((lambda tq=tq, g=g, xts=xts: proj_unit(tq, g, xts)))
            if j >= 1:
                for g in range(8):
                    fillers.append((lambda j=j, g=g: outproj_unit(j - 1, g)))
            for h in range(HPC):
                attn_chain(h, j, fillers)
                if h == 3 and tq + 1 < NQ:
                    load_x(tq + 1)
            while fillers:
                fillers.pop(0)()
            if tq == NQ:
                for g in range(8):
                    outproj_unit(j, g)
    nc.finalize()
    return nc


def _prep_inputs(x, w_qkv, w_out):
    """Shard + pack host-side: returns in_maps for cores 0..7 (core = 2*b + g)."""
    in_maps = []
    maskneg = np.where(
        np.arange(128)[None, :] >= np.arange(128)[:, None], 1.0, 0.0
    ).astype(ml_dtypes.bfloat16)
    for b in range(B):
        xT = np.ascontiguousarray(x[b].T)  # [C, T]
        xq_bf = np.ascontiguousarray(
            xT.reshape(NCHUNK, 128, NQ, TQ).transpose(2, 0, 1, 3)
        ).astype(ml_dtypes.bfloat16)  # [NQ, NCHUNK, 128, TQ]
        for g in range(2):
            wq = w_qkv[:, CPC * g:CPC * (g + 1)]
            wk = w_qkv[:, C + CPC * g:C + CPC * (g + 1)]
            wv = w_qkv[:, 2 * C + CPC * g:2 * C + CPC * (g + 1)]
            wqkv_pack = np.concatenate([wq, wk, wv], axis=1).reshape(
                NCHUNK, 128, 3 * CPC
            )
            wo_pack = np.ascontiguousarray(
                w_out[CPC * g:CPC * (g + 1), :].reshape(NCT, 128, C)
            )
            in_maps.append({
                "wqkv": np.ascontiguousarray(wqkv_pack).astype(ml_dtypes.bfloat16),
                "xq": xq_bf,
                "wo": wo_pack.astype(ml_dtypes.bfloat16),
                "maskneg": maskneg,
            })
    return in_maps


def run(x, w_qkv, w_out, trace=False, trace_cores=None):
    global _NC_CACHE
    if _NC_CACHE is None:
        _NC_CACHE = _build_nc()
    in_maps = _prep_inputs(x, w_qkv, w_out)
    res = run_bass_kernel_spmd(
        _NC_CACHE, in_maps, list(range(8)),
        trace=trace, trace_cores=trace_cores,
    )
    outs = [res.results[i]["out"] for i in range(8)]
    full = np.empty((B, T, C), np.float32)
    for b in range(B):
        full[b] = outs[2 * b] + outs[2 * b + 1]
    return full, res


def kernel(x, w_qkv, w_out):
    x = np.asarray(x, np.float32)
    w_qkv = np.asarray(w_qkv, np.float32)
    w_out = np.asarray(w_out, np.float32)
    full, _ = run(x, w_qkv, w_out)
    return full



# revision 17
# speedup vs baseline: 1.2183x; 1.2183x over previous
"""Causal multi-head attention on 8 Trainium2 NeuronCores.

Problem: x[4,2048,1024] @ w_qkv[1024,3072] -> causal MHA (16 heads, hd=64) -> @ w_out.

Sharding: batch (4) x head-group (2 x 8 heads) = 8 cores. Each core:
  phase 1: QKV projection for its batch + its 8 heads.
           Q^T,K^T produced transposed [ch, t] (lhsT = w chunk, rhs = x^T chunk),
           V produced natural [t, ch] with a ones column per head ([V|1]) so the
           AV matmul also accumulates the softmax denominator.
  phase 2: causal attention per head. S^T[k,q] blocks packed 2-4 key-blocks wide
           into [128,1536]/[128,1024] PSUM tiles so each exp (ACT) instruction
           covers ~1280 columns (amortizes the ~300ns ACTIVATE fixed cost),
           0/1 mask multiply on the diagonal blocks post-exp, attn^T[d,q] +
           denom accumulated in PSUM (AV lags S by 2 groups), normalized by
           reciprocal + gpsimd partition_broadcast + DVE multiply.
  phase 3: partial output projection out = attn^T.T @ w_out (rows of w_out owned
           by this head group). Host sums the two partials per batch.

Schedule notes (PE stream must stay dense — it streams at 216ns/512-col matmul
only when not blocked):
  - DMAs are consolidated (one per x quarter, 2 for QK weights, 1 each wv/wo,
    1 out-DMA per 128-row tile) because each DMA_DIRECT2D costs ~650ns of Sync
    engine issue time; with 8+ chunk DMAs the first weights arrive ~5us late.
  - only Q0/K0 + V projection run before the attention chains; the remaining
    projection groups interleave as fillers inside the j=0 chains.
  - output projection for quarter q fills at j=q+2 (quarters 1,2 at j=3) where
    the causal attention is ACT(exp)-bound and the PE has idle slots.
  - fillers pop paced (stride) so they cover the whole j-loop including the
    last chain, keeping the PE warm through the final normalize.

Measured ~284-285us on hardware (max core, warm), rel err 3.5e-3 vs the fp32
reference (baseline schedule was ~294us; PE column-stream floor is ~225us).
"""
import sys

if "/opt/trn_rl_repo" not in sys.path:
    sys.path.insert(0, "/opt/trn_rl_repo")

import ml_dtypes
import numpy as np

import concourse.tile as tile
from concourse import bacc, mybir
from concourse.bass_utils import run_bass_kernel_spmd

F32 = mybir.dt.float32
BF16 = mybir.dt.bfloat16
EXP = mybir.ActivationFunctionType.Exp

B, T, C, H = 4, 2048, 1024, 16
HD = C // H              # 64
HPC = 8                  # heads per core
CPC = HPC * HD           # 512 channels per core
NCHUNK = C // 128        # 8 contraction chunks of 128
NQ = 4                   # t-quarters (512 each)
TQ = T // NQ             # 512
NKB = T // 128           # 16 key blocks
NCT = CPC // 128         # 4 c'-tiles per projection (q and k each)
VSTRIDE = CPC + HPC      # 520: per key-block [V|1] packing width
GORDER = [0, 4, 1, 5, 2, 6, 3, 7]   # QK group DMA order: Q0,K0 first

_NC_CACHE = None


def _segments(j):
    """Causal S^T segments for 512-query tile j: (kb, col0, n) per key block."""
    segs = []
    for kb in range(4 * j + 4):
        col0 = 0 if kb < 4 * j else 128 * (kb - 4 * j)
        segs.append((kb, col0, 512 - col0))
    return segs


def _pack_groups(segs, caps):
    """Pack segments into PSUM groups of alternating capacity, hole-free.

    A matmul's output must stay inside one 512-col PSUM bank, and the exp
    reads the group's [0:wsum] span, so segments are first-fit reordered
    into 512-col banks with no gaps (asserted). Returns a list of
    (cap_idx, [(kb, col0, n, off)], wsum)."""
    groups = []
    remaining = list(segs)
    ci = 0
    while remaining:
        cap = caps[ci % len(caps)]
        cur, off = [], 0
        while remaining and off < cap:
            bank_rem = 512 - off % 512
            pick = None
            for s in remaining:
                if s[2] <= bank_rem and off + s[2] <= cap:
                    pick = s
                    break
            if pick is None:
                break
            remaining.remove(pick)
            cur.append((pick[0], pick[1], pick[2], off))
            off += pick[2]
        assert cur, f"segment does not fit: {remaining[0]} cap={cap} off={off}"
        groups.append((ci % len(caps), cur, off))
        ci += 1
    return groups


def _build_nc():
    nc = bacc.Bacc()

    wqk = nc.dram_tensor("wqk", [128, 2 * NCT * NCHUNK * 128], BF16,
                         kind="ExternalInput")
    wv = nc.dram_tensor("wv", [128, NCHUNK * CPC], BF16, kind="ExternalInput")
    xq = nc.dram_tensor("xq", [NQ, 128, NCHUNK * TQ], BF16, kind="ExternalInput")
    wo = nc.dram_tensor("wo", [128, NCT * C], BF16, kind="ExternalInput")
    maskneg = nc.dram_tensor("maskneg", [128, 128], BF16, kind="ExternalInput")
    out = nc.dram_tensor("out", [T, C], F32, kind="ExternalOutput")

    CAPS = (1536, 1024)
    SS_TAGS = ("S0", "S1")
    GPOS = {g: GORDER.index(g) for g in range(2 * NCT)}

    with tile.TileContext(nc) as tc, \
         tc.tile_pool(name="pers", bufs=1) as pers, \
         tc.tile_pool(name="xpool", bufs=2) as xpool, \
         tc.tile_pool(name="epool", bufs=4) as epool, \
         tc.tile_pool(name="npool", bufs=4) as npool, \
         tc.tile_pool(name="opool", bufs=2) as opool, \
         tc.tile_pool(name="psum", bufs=1, space="PSUM") as psum:
        # persistent SBUF
        qkt = [pers.tile([128, T], BF16, name=f"qkt{i}") for i in range(2 * NCT)]
        vsb = pers.tile([128, NKB * VSTRIDE], BF16, name="vsb")
        atn = [pers.tile([128, T], BF16, name=f"atn{i}") for i in range(NCT)]
        mask_sb = pers.tile([128, 128], BF16, name="mask_sb")
        # ones columns of [V|1]
        ones_sb = pers.tile([128, NKB * HPC], F32, name="ones_sb")
        nc.vector.memset(ones_sb[:], 1.0)
        nc.vector.tensor_copy(
            vsb.rearrange("p (t h e) -> p (t h) e", h=HPC, e=HD + 1)[:, :, HD:HD + 1],
            ones_sb[:, :, None],
        )

        # ---- consolidated DMAs: x q0 + first QK groups first ----
        xt_cur = [None]

        def load_x(tq, split=False):
            xt = xpool.tile([128, NCHUNK * TQ], BF16, name="xfull", tag="x")
            if split:
                nc.sync.dma_start(xt[:, 0:NCHUNK * TQ // 2],
                                  xq[tq, :, 0:NCHUNK * TQ // 2])
                nc.sync.dma_start(xt[:, NCHUNK * TQ // 2:],
                                  xq[tq, :, NCHUNK * TQ // 2:])
            else:
                nc.sync.dma_start(xt[:], xq[tq])
            xt_cur[0] = xt

        # weights first: DMA completions are in issue order with ~4us pipeline
        # latency, and the first projection group needs Q0 weights + x chunks
        # in stream order — split the gating transfers small so the first
        # matmul starts as early as possible.
        wqk_all = pers.tile([128, 2 * NCT * NCHUNK * 128], BF16, name="wqk_all")
        nc.sync.dma_start(wqk_all[:, 0:1024], wqk[:, 0:1024])          # g0
        xt = xpool.tile([128, NCHUNK * TQ], BF16, name="xfull", tag="x")
        nc.sync.dma_start(xt[:, 0:1024], xq[0, :, 0:1024])             # c0,c1
        nc.sync.dma_start(xt[:, 1024:2048], xq[0, :, 1024:2048])       # c2,c3
        nc.sync.dma_start(wqk_all[:, 1024:2048], wqk[:, 1024:2048])    # g4
        nc.sync.dma_start(xt[:, 2048:4096], xq[0, :, 2048:4096])       # c4-7
        xt_cur[0] = xt
        wv_all = pers.tile([128, NCHUNK * CPC], BF16, name="wv_all")
        nc.sync.dma_start(wv_all[:], wv[:, :])
        nc.sync.dma_start(wqk_all[:, 2048:], wqk[:, 2048:])
        wo_all = pers.tile([128, NCT * C], BF16, name="wo_all")
        nc.sync.dma_start(wo_all[:], wo[:, :])
        nc.sync.dma_start(mask_sb[:], maskneg[:, :])

        def wqk_sb(g):
            return wqk_all[:, 1024 * GPOS[g]:1024 * (GPOS[g] + 1)]

        # ---- unit emitters ----
        def proj_qk(tq, g, xt, ptag="P"):
            ps = psum.tile([128, TQ], F32, name="psqk", tag=ptag, bufs=1)
            w = wqk_sb(g)
            for c in range(NCHUNK):
                nc.tensor.matmul(
                    ps[:], w[:, 128 * c:128 * (c + 1)],
                    xt[:, TQ * c:TQ * c + TQ],
                    start=(c == 0), stop=(c == NCHUNK - 1),
                )
            nc.vector.tensor_copy(qkt[g][:, TQ * tq:TQ * (tq + 1)], ps[:])

        def proj_v(tq, vt, xt, ptag="P"):
            ps = psum.tile([128, CPC], F32, name="psv", tag=ptag, bufs=1)
            for c in range(NCHUNK):
                nc.tensor.matmul(
                    ps[:], xt[:, TQ * c + 128 * vt:TQ * c + 128 * (vt + 1)],
                    wv_all[:, CPC * c:CPC * (c + 1)],
                    start=(c == 0), stop=(c == NCHUNK - 1),
                )
            ti = tq * (TQ // 128) + vt
            dst = vsb[:, VSTRIDE * ti:VSTRIDE * (ti + 1)]
            nc.vector.tensor_copy(
                dst.rearrange("p (h e) -> p h e", e=HD + 1)[:, :, 0:HD],
                ps.rearrange("p (h e) -> p h e", e=HD),
            )

        oc_by_tt = {}

        def outproj(tt, jj, ptag="P"):
            """Half-unit: [128 t-rows, 512 out-cols]; DMA fires on jj==1."""
            ps = psum.tile([128, 512], F32, name="po", tag=ptag, bufs=1)
            for cc in range(NCT):
                nc.tensor.matmul(
                    ps[:], atn[cc][:, 128 * tt:128 * (tt + 1)],
                    wo_all[:, C * cc + 512 * jj:C * cc + 512 * (jj + 1)],
                    start=(cc == 0), stop=(cc == NCT - 1),
                )
            if tt not in oc_by_tt:
                oc_by_tt[tt] = opool.tile([128, C], F32, name="oc", tag="oc")
            oc = oc_by_tt[tt]
            nc.vector.tensor_copy(oc[:, 512 * jj:512 * (jj + 1)], ps[:])
            if jj == 1:
                nc.sync.dma_start(out[128 * tt:128 * (tt + 1), :], oc[:])
                del oc_by_tt[tt]

        ss_flip = [0]

        def attn_chain(h, j, fillers, pace):
            """Causal attention for (head h, 512-query tile j)."""
            ct, r0 = h // 2, HD * (h % 2)
            Qh = qkt[ct][r0:r0 + HD, :]
            Kh = qkt[NCT + ct][r0:r0 + HD, :]
            q0 = 512 * j
            nkb = 4 * j + 4
            caps = (CAPS[ss_flip[0] % 2], CAPS[(ss_flip[0] + 1) % 2])
            groups = _pack_groups(_segments(j), caps)
            pa = psum.tile([HD + 1, 512], F32, name="pa", tag="A", bufs=2)
            ees = [None] * len(groups)

            def pop_filler():
                pace[0] += 1
                if fillers and pace[0] >= pace[1]:
                    pace[0] = 0
                    fillers.pop(0)()

            def emit_s(gi):
                ci, segs, wsum = groups[gi]
                ss = psum.tile([128, caps[ci]], F32, name=f"ss{ci}",
                               tag=SS_TAGS[(ss_flip[0] + ci) % 2], bufs=1)
                ee = epool.tile([128, caps[ci]], BF16, name="ee", tag="E")
                for kb, col0, n, off in segs:
                    nc.tensor.matmul(
                        ss[:, off:off + n],
                        Kh[:, 128 * kb:128 * (kb + 1)],
                        Qh[:, q0 + col0:q0 + 512],
                        start=True, stop=True, skip_group_check=True,
                    )
                nc.scalar.activation(ee[:, 0:wsum], ss[:, 0:wsum], EXP, scale=0.125)
                for kb, col0, n, off in segs:
                    if kb >= 4 * j:  # zero the masked (future) triangle post-exp
                        nc.vector.tensor_mul(
                            ee[:, off:off + 128], ee[:, off:off + 128], mask_sb[:]
                        )
                ees[gi] = ee

            def emit_a(gi):
                _, segs, _ = groups[gi]
                ee = ees[gi]
                for kb, col0, n, off in segs:
                    nc.tensor.matmul(
                        pa[:, col0:512],
                        vsb[:, VSTRIDE * kb + (HD + 1) * h:
                             VSTRIDE * kb + (HD + 1) * (h + 1)],
                        ee[:, off:off + n],
                        start=(kb == 0), stop=(kb == nkb - 1),
                        skip_group_check=True,
                    )

            ngrp = len(groups)
            LAG = min(2, ngrp)
            for gi in range(ngrp):
                emit_s(gi)
                pop_filler()
                if gi >= LAG:
                    emit_a(gi - LAG)
                    pop_filler()
            for gi in range(ngrp - LAG, ngrp):
                emit_a(gi)
                pop_filler()
            ss_flip[0] = (ss_flip[0] + ngrp) % 2

            # normalize by the denominator row
            den = npool.tile([1, 512], F32, name="den", tag="den")
            nc.vector.tensor_copy(den[:], pa[HD:HD + 1, :])
            rec = npool.tile([1, 512], F32, name="rec", tag="rec")
            nc.vector.reciprocal_approx_fast(rec[:], den[:])
            bc = npool.tile([HD, 512], F32, name="bc", tag="bc")
            nc.gpsimd.partition_broadcast(bc[:], rec[:])
            nc.vector.tensor_mul(
                atn[ct][r0:r0 + HD, q0:q0 + 512], pa[0:HD, :], bc[:]
            )

        # ---- schedule ----
        # minimal initial projection: Q0/K0 tiles + quarter-0 V, then chains
        # start; remaining quarter-0 QK groups lead the j=0 filler list in
        # the order heads need them (h=2k needs group k).
        ptags = ["P", "S0", "S1"]
        xt0 = xt_cur[0]
        proj_qk(0, 0, xt0, ptag="P")
        proj_qk(0, NCT, xt0, ptag="S0")
        for vt in range(NCT):
            proj_v(0, vt, xt0, ptag=ptags[(2 + vt) % 3])
        load_x(1)

        # groups-per-j for pacing (approximate; flip shifts it by ~1)
        for tq in range(1, NQ + 1):
            j = tq - 1
            fillers = []
            if j == 0:
                for g in (1, NCT + 1, 2, NCT + 2, 3, NCT + 3):
                    fillers.append(
                        lambda g=g, xts=xt0: proj_qk(0, g, xts))
            if tq < NQ:
                xts = xt_cur[0]
                for g in range(2 * NCT):
                    fillers.append(lambda tq=tq, g=g, xts=xts: proj_qk(tq, g, xts))
                for vt in range(NCT):
                    fillers.append(lambda tq=tq, vt=vt, xts=xts: proj_v(tq, vt, xts))
            # output projection: all done quarters fill at j=3 where the
            # attention inner loop is ACT(exp)-bound and the PE has slack
            oquarters = {3: [0, 1, 2]}.get(j, [])
            for oq in oquarters:
                for g in range(8):
                    fillers.append(
                        lambda oq=oq, g=g: outproj(4 * oq + g // 2, g % 2))
            nslots = 16 * len(_pack_groups(_segments(j), CAPS))
            stride = max(1, nslots // max(1, len(fillers)))
            pace = [stride - 1, stride]  # counter, stride: pop on first slot
            for h in range(HPC):
                attn_chain(h, j, fillers, pace)
                if h == 3 and tq + 1 < NQ:
                    load_x(tq + 1)
            while fillers:
                fillers.pop(0)()
            if tq == NQ:
                # final quarter: open each half's PSUM group with the cc=0..2
                # matmuls (independent of the last heads' normalize) so the PE
                # stays busy through it; only cc=3 + copy + DMA trail.
                halves = [(12 + g // 2, g % 2) for g in range(8)]
                pss = {}

                def open_half(i):
                    tt, jj = halves[i]
                    ps = psum.tile([128, 512], F32, name="po",
                                   tag=ptags[i % 3], bufs=1)
                    pss[i] = ps
                    for cc in range(NCT - 1):
                        nc.tensor.matmul(
                            ps[:], atn[cc][:, 128 * tt:128 * (tt + 1)],
                            wo_all[:, C * cc + 512 * jj:C * cc + 512 * (jj + 1)],
                            start=(cc == 0), stop=False,
                        )

                for i in range(3):
                    open_half(i)
                for i in range(8):
                    tt, jj = halves[i]
                    ps = pss.pop(i)
                    cc = NCT - 1
                    nc.tensor.matmul(
                        ps[:], atn[cc][:, 128 * tt:128 * (tt + 1)],
                        wo_all[:, C * cc + 512 * jj:C * cc + 512 * (jj + 1)],
                        start=False, stop=True,
                    )
                    if i + 3 < 8:
                        open_half(i + 3)
                    if tt not in oc_by_tt:
                        oc_by_tt[tt] = opool.tile([128, C], F32, name="oc",
                                                  tag="oc")
                    oc = oc_by_tt[tt]
                    nc.vector.tensor_copy(oc[:, 512 * jj:512 * (jj + 1)], ps[:])
                    # per-half DMA at the tail: the last transfer is smaller
                    # and issues earlier than a combined per-tt DMA would
                    nc.sync.dma_start(
                        out[128 * tt:128 * (tt + 1), 512 * jj:512 * (jj + 1)],
                        oc[:, 512 * jj:512 * (jj + 1)],
                    )
                    if jj == 1:
                        del oc_by_tt[tt]
    nc.finalize()
    return nc


def _prep_inputs(x, w_qkv, w_out):
    """Shard + pack host-side: returns in_maps for cores 0..7 (core = 2*b + g)."""
    in_maps = []
    maskneg = np.where(
        np.arange(128)[None, :] >= np.arange(128)[:, None], 1.0, 0.0
    ).astype(ml_dtypes.bfloat16)
    for b in range(B):
        xT = np.ascontiguousarray(x[b].T)  # [C, T]
        # [NQ, 128, NCHUNK*TQ]: quarter tq, partition p, col 512c+f
        xq_bf = np.ascontiguousarray(
            xT.reshape(NCHUNK, 128, NQ, TQ).transpose(2, 1, 0, 3)
            .reshape(NQ, 128, NCHUNK * TQ)
        ).astype(ml_dtypes.bfloat16)
        for g in range(2):
            wq = w_qkv[:, CPC * g:CPC * (g + 1)]
            wk = w_qkv[:, C + CPC * g:C + CPC * (g + 1)]
            wvm = w_qkv[:, 2 * C + CPC * g:2 * C + CPC * (g + 1)]
            # QK tiles in GORDER, each [128, NCHUNK*128] with [p, 128c+f]
            qk_tiles = [
                wq.reshape(NCHUNK, 128, CPC)[:, :, 128 * t:128 * (t + 1)]
                .transpose(1, 0, 2).reshape(128, NCHUNK * 128)
                for t in range(NCT)
            ] + [
                wk.reshape(NCHUNK, 128, CPC)[:, :, 128 * t:128 * (t + 1)]
                .transpose(1, 0, 2).reshape(128, NCHUNK * 128)
                for t in range(NCT)
            ]
            wqk_pack = np.concatenate([qk_tiles[g2] for g2 in GORDER], axis=1)
            # V: [128, NCHUNK*CPC] with [p, CPC*c+f]
            wv_pack = np.ascontiguousarray(
                wvm.reshape(NCHUNK, 128, CPC).transpose(1, 0, 2)
                .reshape(128, NCHUNK * CPC)
            )
            # wo: [128, NCT*C] with [p, C*cc+f]
            wo_pack = np.ascontiguousarray(
                w_out[CPC * g:CPC * (g + 1), :].reshape(NCT, 128, C)
                .transpose(1, 0, 2).reshape(128, NCT * C)
            )
            in_maps.append({
                "wqk": np.ascontiguousarray(wqk_pack).astype(ml_dtypes.bfloat16),
                "wv": wv_pack.astype(ml_dtypes.bfloat16),
                "xq": xq_bf,
                "wo": wo_pack.astype(ml_dtypes.bfloat16),
                "maskneg": maskneg,
            })
    return in_maps


def run(x, w_qkv, w_out, trace=False, trace_cores=None):
    global _NC_CACHE
    if _NC_CACHE is None:
        _NC_CACHE = _build_nc()
    in_maps = _prep_inputs(x, w_qkv, w_out)
    res = run_bass_kernel_spmd(
        _NC_CACHE, in_maps, list(range(8)),
        trace=trace, trace_cores=trace_cores,
    )
    outs = [res.results[i]["out"] for i in range(8)]
    full = np.empty((B, T, C), np.float32)
    for b in range(B):
        full[b] = outs[2 * b] + outs[2 * b + 1]
    return full, res


def kernel(x, w_qkv, w_out):
    x = np.asarray(x, np.float32)
    w_qkv = np.asarray(w_qkv, np.float32)
    w_out = np.asarray(w_out, np.float32)
    full, _ = run(x, w_qkv, w_out)
    return full
